# revision 15
# baseline (speedup 1.0000x reference)
"""CoxTime loss kernel for 8 Trainium2 NeuronCores.

Strategy (data-parallel over B):
  Each core reduces its (32768, 128) f32 logits shard to a (128, 128)
  binned summary using the TensorEngine with an on-the-fly one-hot of
  labels:
      S[c, k] = sum_{j: label_j == c} exp(logits[j, k])
  Layout [P, NT] (row j = p*NT + t) keeps every DMA partition-line
  contiguous AND gives per-partition label columns, so the one-hot is
  a single-source tensor_scalar(is_equal) per row-tile — the DVE's 4x
  mode — instead of a broadcast tensor_tensor (1x).  exp runs on the
  scalar engine.  The label mask (labels >= k) is applied by the
  matmul binning plus a host-side triangular sum.  The iota row and
  labels are DMA'd in as bf16 constants.  The host all-reduces the 8
  summaries and finishes with the O(B) 1-D work: numer/n_ev from
  labels+events+own logits, the log, and the scalar reduction.
"""

import ml_dtypes
import numpy as np

import concourse.bacc as bacc
import concourse.bass as bass
import concourse.mybir as mybir
import concourse.tile as tile
from concourse.bass_utils import run_bass_kernel_spmd

B = 262144
K = 128
NCORES = 8
BC = B // NCORES  # rows per core
P = 128           # partitions
NT = BC // P      # row-tiles per core (column index t in the [P, NT] view)
NBANK = 8         # PSUM banks rotated across row-tiles for matmul ILP

TPB = 16          # row-tiles per DMA'd big tile
HPB = TPB // 2    # row-tiles per exp/one-hot chunk (finer pipeline grain)

f32 = mybir.dt.float32
bf16 = mybir.dt.bfloat16

LAST_EXEC_NS = None
LAST_TRACE = None
LAST_PROFILE_JSON = None


def build_nc():
    """Build the per-core Bass program."""
    nc = bacc.Bacc("TRN2", target_bir_lowering=False)
    logits = nc.declare_dram_parameter("logits", [BC, K], f32, isOutput=False)
    labcols = nc.declare_dram_parameter("labcols", [P, NT], f32,
                                        isOutput=False)
    iotak = nc.declare_dram_parameter("iotak", [P, K], bf16, isOutput=False)
    out = nc.declare_dram_parameter("out", [P, NBANK * K], f32, isOutput=True)

    with tile.TileContext(nc) as tc:
        with (
            tc.tile_pool(name="const", bufs=1) as cpool,
            tc.tile_pool(name="lt", bufs=6) as ltpool,
            tc.tile_pool(name="ee", bufs=8) as epool,
            tc.tile_pool(name="oh", bufs=8) as ohpool,
            tc.tile_pool(name="psum", bufs=1, space="PSUM") as pspool,
        ):
            # first logits DMA goes out before the tiny const DMAs
            lg3 = logits.ap().rearrange("(p t) k -> p t k", p=P)
            lt_first = ltpool.tile([P, TPB * K], f32)
            nc.sync.dma_start(out=lt_first[:], in_=lg3[:, 0:TPB, :])

            labc = cpool.tile([P, NT], f32)
            nc.sync.dma_start(out=labc[:], in_=labcols.ap())
            iota = cpool.tile([P, K], bf16)
            nc.sync.dma_start(out=iota[:], in_=iotak.ap())

            psums = [pspool.tile([P, K], f32, name=f"ps{b}", tag=f"ps{b}")
                     for b in range(NBANK)]
            osb = cpool.tile([P, NBANK * K], f32)

            HW = HPB * K
            NSEG = NT // TPB
            for g in range(NSEG):
                t0 = g * TPB
                last = g == NSEG - 1
                if g == 0:
                    lt = lt_first
                elif not last:
                    lt = ltpool.tile([P, TPB * K], f32)
                    eng = nc.sync if g % 2 == 0 else nc.scalar
                    eng.dma_start(out=lt[:], in_=lg3[:, t0:t0 + TPB, :])

                # chunked: finer exp grain keeps the matmul stream right
                # behind the DMA; the last seg uses quarter chunks, each
                # with its own sub-DMA, so the post-DMA drain is short
                cpb = HPB if not last else HPB // 2
                for h in range(TPB // cpb):
                    h0 = t0 + h * cpb
                    cw = cpb * K

                    if last:
                        lt = ltpool.tile([P, cw], f32)
                        nc.sync.dma_start(out=lt[:],
                                          in_=lg3[:, h0:h0 + cpb, :])
                        src_ap = lt[:]
                    else:
                        src_ap = lt[:, h * cw:(h + 1) * cw]

                    # E = exp(logits), cast to bf16
                    ee = epool.tile([P, cw], bf16)
                    nc.scalar.activation(
                        out=ee[:], in_=src_ap,
                        func=mybir.ActivationFunctionType.Exp)

                    # one-hot per row-tile: oh[p, k] = (label[p, t] == k)
                    # (single-source tensor_scalar -> fast DVE mode)
                    oh = ohpool.tile([P, cw], bf16)
                    for q in range(cpb):
                        t = h0 + q
                        nc.vector.tensor_scalar(
                            out=oh[:, q * K:(q + 1) * K], in0=iota[:],
                            scalar1=labc[:, t:t + 1], scalar2=None,
                            op0=mybir.AluOpType.is_equal)

                    # bin exp by label: S[c, k] += sum_p oh[p,c] * E[p,k]
                    for q in range(cpb):
                        t = h0 + q
                        b = t % NBANK
                        nc.tensor.matmul(
                            out=psums[b][:],
                            lhsT=oh[:, q * K:(q + 1) * K],
                            rhs=ee[:, q * K:(q + 1) * K],
                            start=(t < NBANK),
                            stop=(t >= NT - NBANK),
                        )

            for b in range(NBANK):
                if b % 2 == 0:
                    nc.vector.tensor_copy(osb[:, b * K:(b + 1) * K],
                                          psums[b][:])
                else:
                    nc.scalar.copy(osb[:, b * K:(b + 1) * K], psums[b][:])
            nc.sync.dma_start(out=out.ap(), in_=osb[:])

    nc.compile()
    return nc


def _shard_inputs(logits, labels):
    """Build the 8 per-core input maps (host-side layout only)."""
    logits = np.ascontiguousarray(np.asarray(logits, dtype=np.float32))
    labels = np.asarray(labels, dtype=np.int32)
    iota = np.broadcast_to(np.arange(K, dtype=np.float32), (P, K)).astype(
        ml_dtypes.bfloat16)
    in_maps = []
    for i in range(NCORES):
        sl = slice(i * BC, (i + 1) * BC)
        lab = labels[sl].astype(np.float32).reshape(P, NT)
        in_maps.append({
            "logits": logits[sl],
            "labcols": np.ascontiguousarray(lab),
            "iotak": iota,
        })
    return in_maps


def _finish(outs, logits, labels, events):
    """Host epilogue: all-reduce binned sums, triangular sum, numer/n_ev
    from 1-D data, the log, and the final scalar reduction."""
    labels = np.asarray(labels, dtype=np.int32)
    events = np.asarray(events, dtype=np.int32)
    S = np.zeros((P, K), dtype=np.float64)
    for o in outs:
        S += o.astype(np.float64).reshape(P, NBANK, K).sum(axis=1)
    # sumexp[k] = sum over label bins c >= k
    sumexp = (S * np.tri(K)).sum(axis=0)
    ev = events == 1
    own = np.asarray(logits)[np.arange(labels.shape[0]), labels].astype(
        np.float64)
    n_ev = np.bincount(labels[ev], minlength=K).astype(np.float64)
    numer = np.bincount(labels[ev], weights=own[ev], minlength=K)
    with np.errstate(divide="ignore"):
        denom_log = np.log(sumexp)
    terms = np.where(n_ev > 0, numer - n_ev * denom_log, 0.0)
    n_total = max(n_ev.sum(), 1.0)
    return np.array(-terms.sum() / n_total, dtype=np.float32)


def kernel(logits, labels, events, _trace=False):
    global LAST_EXEC_NS, LAST_TRACE, LAST_PROFILE_JSON
    in_maps = _shard_inputs(logits, labels)
    nc = build_nc()
    try:
        res = run_bass_kernel_spmd(nc, in_maps, core_ids=list(range(NCORES)),
                                   trace=_trace)
    except Exception:
        # one retry: absorbs transient NRT device-unrecoverable hiccups
        res = run_bass_kernel_spmd(nc, in_maps, core_ids=list(range(NCORES)),
                                   trace=_trace)
    LAST_EXEC_NS = res.exec_time_ns
    LAST_TRACE = res.instructions_and_trace
    LAST_PROFILE_JSON = res.profile_json
    outs = [res.results[i]["out"] for i in range(NCORES)]
    return _finish(outs, logits, labels, events)


# revision 16
# speedup vs baseline: 1.0859x; 1.0859x over previous
"""CoxTime loss kernel for 8 Trainium2 NeuronCores.

Strategy (data-parallel over B):
  Each core reduces its (32768, 128) f32 logits shard to a (128, 128)
  binned summary using the TensorEngine with an on-the-fly one-hot of
  labels:
      S[c, k] = sum_{j: label_j == c} exp(logits[j, k])
  Layout [P, NT] (row j = p*NT + t) keeps every DMA partition-line
  contiguous AND gives per-partition label columns, so the one-hot is
  a single-source tensor_scalar(is_equal) per row-tile — the DVE's 4x
  mode — instead of a broadcast tensor_tensor (1x).  exp runs on the
  scalar engine.  The label mask (labels >= k) is applied by the
  matmul binning plus a host-side triangular sum.  The iota row and
  labels are DMA'd in as bf16 constants.  The host all-reduces the 8
  summaries and finishes with the O(B) 1-D work: numer/n_ev from
  labels+events+own logits, the log, and the scalar reduction.
"""

import ml_dtypes
import numpy as np

import concourse.bacc as bacc
import concourse.bass as bass
import concourse.mybir as mybir
import concourse.tile as tile
from concourse.bass_utils import run_bass_kernel_spmd

B = 262144
K = 128
NCORES = 8
BC = B // NCORES  # rows per core
P = 128           # partitions
NT = BC // P      # row-tiles per core (column index t in the [P, NT] view)
NBANK = 8         # PSUM banks rotated across row-tiles for matmul ILP

TPB = 16          # row-tiles per DMA'd big tile
HPB = TPB // 2    # row-tiles per exp/one-hot chunk (finer pipeline grain)

f32 = mybir.dt.float32
bf16 = mybir.dt.bfloat16

LAST_EXEC_NS = None
LAST_TRACE = None
LAST_PROFILE_JSON = None


def build_nc():
    """Build the per-core Bass program."""
    nc = bacc.Bacc("TRN2", target_bir_lowering=False)
    logits = nc.declare_dram_parameter("logits", [BC, K], f32, isOutput=False)
    labcols = nc.declare_dram_parameter("labcols", [P, NT], f32,
                                        isOutput=False)
    iotak = nc.declare_dram_parameter("iotak", [P, K], bf16, isOutput=False)
    out = nc.declare_dram_parameter("out", [P, NBANK * K], f32, isOutput=True)

    with tile.TileContext(nc) as tc:
        with (
            tc.tile_pool(name="const", bufs=1) as cpool,
            tc.tile_pool(name="lt", bufs=6) as ltpool,
            tc.tile_pool(name="ee", bufs=8) as epool,
            tc.tile_pool(name="oh", bufs=8) as ohpool,
            tc.tile_pool(name="psum", bufs=1, space="PSUM") as pspool,
        ):
            # first logits DMA goes out before the tiny const DMAs
            lg3 = logits.ap().rearrange("(p t) k -> p t k", p=P)
            lt_first = ltpool.tile([P, TPB * K], f32)
            nc.sync.dma_start(out=lt_first[:], in_=lg3[:, 0:TPB, :])

            labc = cpool.tile([P, NT], f32)
            nc.sync.dma_start(out=labc[:], in_=labcols.ap())
            iota = cpool.tile([P, K], bf16)
            nc.sync.dma_start(out=iota[:], in_=iotak.ap())

            psums = [pspool.tile([P, K], f32, name=f"ps{b}", tag=f"ps{b}")
                     for b in range(NBANK)]
            osb = cpool.tile([P, NBANK * K], f32)

            HW = HPB * K
            NSEG = NT // TPB
            for g in range(NSEG):
                t0 = g * TPB
                last = g == NSEG - 1
                if g == 0:
                    lt = lt_first
                elif not last:
                    lt = ltpool.tile([P, TPB * K], f32)
                    nc.sync.dma_start(out=lt[:], in_=lg3[:, t0:t0 + TPB, :])

                # chunked: finer exp grain keeps the matmul stream right
                # behind the DMA; the last seg uses quarter chunks, each
                # with its own sub-DMA, so the post-DMA drain is short
                cpb = HPB if not last else HPB // 2
                for h in range(TPB // cpb):
                    h0 = t0 + h * cpb
                    cw = cpb * K

                    if last:
                        lt = ltpool.tile([P, cw], f32)
                        nc.sync.dma_start(out=lt[:],
                                          in_=lg3[:, h0:h0 + cpb, :])
                        src_ap = lt[:]
                    else:
                        src_ap = lt[:, h * cw:(h + 1) * cw]

                    # E = exp(logits), cast to bf16
                    ee = epool.tile([P, cw], bf16)
                    nc.scalar.activation(
                        out=ee[:], in_=src_ap,
                        func=mybir.ActivationFunctionType.Exp)

                    # one-hot per row-tile: oh[p, k] = (label[p, t] == k)
                    # (single-source tensor_scalar -> fast DVE mode)
                    oh = ohpool.tile([P, cw], bf16)
                    for q in range(cpb):
                        t = h0 + q
                        nc.vector.tensor_scalar(
                            out=oh[:, q * K:(q + 1) * K], in0=iota[:],
                            scalar1=labc[:, t:t + 1], scalar2=None,
                            op0=mybir.AluOpType.is_equal)

                    # bin exp by label: S[c, k] += sum_p oh[p,c] * E[p,k]
                    for q in range(cpb):
                        t = h0 + q
                        b = t % NBANK
                        nc.tensor.matmul(
                            out=psums[b][:],
                            lhsT=oh[:, q * K:(q + 1) * K],
                            rhs=ee[:, q * K:(q + 1) * K],
                            start=(t < NBANK),
                            stop=(t >= NT - NBANK),
                        )

            for b in range(NBANK):
                if b % 2 == 0:
                    nc.vector.tensor_copy(osb[:, b * K:(b + 1) * K],
                                          psums[b][:])
                else:
                    nc.scalar.copy(osb[:, b * K:(b + 1) * K], psums[b][:])
            nc.sync.dma_start(out=out.ap(), in_=osb[:])

    nc.compile()
    return nc


def _shard_inputs(logits, labels):
    """Build the 8 per-core input maps (host-side layout only)."""
    logits = np.ascontiguousarray(np.asarray(logits, dtype=np.float32))
    labels = np.asarray(labels, dtype=np.int32)
    iota = np.broadcast_to(np.arange(K, dtype=np.float32), (P, K)).astype(
        ml_dtypes.bfloat16)
    in_maps = []
    for i in range(NCORES):
        sl = slice(i * BC, (i + 1) * BC)
        lab = labels[sl].astype(np.float32).reshape(P, NT)
        in_maps.append({
            "logits": logits[sl],
            "labcols": np.ascontiguousarray(lab),
            "iotak": iota,
        })
    return in_maps


def _finish(outs, logits, labels, events):
    """Host epilogue: all-reduce binned sums, triangular sum, numer/n_ev
    from 1-D data, the log, and the final scalar reduction."""
    labels = np.asarray(labels, dtype=np.int32)
    events = np.asarray(events, dtype=np.int32)
    S = np.zeros((P, K), dtype=np.float64)
    for o in outs:
        S += o.astype(np.float64).reshape(P, NBANK, K).sum(axis=1)
    # sumexp[k] = sum over label bins c >= k
    sumexp = (S * np.tri(K)).sum(axis=0)
    ev = events == 1
    own = np.asarray(logits)[np.arange(labels.shape[0]), labels].astype(
        np.float64)
    n_ev = np.bincount(labels[ev], minlength=K).astype(np.float64)
    numer = np.bincount(labels[ev], weights=own[ev], minlength=K)
    with np.errstate(divide="ignore"):
        denom_log = np.log(sumexp)
    terms = np.where(n_ev > 0, numer - n_ev * denom_log, 0.0)
    n_total = max(n_ev.sum(), 1.0)
    return np.array(-terms.sum() / n_total, dtype=np.float32)


def kernel(logits, labels, events, _trace=False):
    global LAST_EXEC_NS, LAST_TRACE, LAST_PROFILE_JSON
    in_maps = _shard_inputs(logits, labels)
    nc = build_nc()
    try:
        res = run_bass_kernel_spmd(nc, in_maps, core_ids=list(range(NCORES)),
                                   trace=_trace)
    except Exception:
        # one retry: absorbs transient NRT device-unrecoverable hiccups
        res = run_bass_kernel_spmd(nc, in_maps, core_ids=list(range(NCORES)),
                                   trace=_trace)
    LAST_EXEC_NS = res.exec_time_ns
    LAST_TRACE = res.instructions_and_trace
    LAST_PROFILE_JSON = res.profile_json
    outs = [res.results[i]["out"] for i in range(NCORES)]
    return _finish(outs, logits, labels, events)


# revision 17
# speedup vs baseline: 1.0959x; 1.0092x over previous
"""CoxTime loss kernel for 8 Trainium2 NeuronCores.

Strategy (data-parallel over B):
  Each core reduces its (32768, 128) f32 logits shard to a (128, 128)
  binned summary using the TensorEngine with an on-the-fly one-hot of
  labels:
      S[c, k] = sum_{j: label_j == c} exp(logits[j, k])
  Layout [P, NT] (row j = p*NT + t) keeps every DMA partition-line
  contiguous AND gives per-partition label columns, so the one-hot is
  a single-source tensor_scalar(is_equal) per row-tile — the DVE's 4x
  mode — instead of a broadcast tensor_tensor (1x).  exp runs on the
  scalar engine.  The label mask (labels >= k) is applied by the
  matmul binning plus a host-side triangular sum.  The iota row and
  labels are DMA'd in as bf16 constants.  The host all-reduces the 8
  summaries and finishes with the O(B) 1-D work: numer/n_ev from
  labels+events+own logits, the log, and the scalar reduction.
"""

import ml_dtypes
import numpy as np

import concourse.bacc as bacc
import concourse.bass as bass
import concourse.mybir as mybir
import concourse.tile as tile
from concourse.bass_utils import run_bass_kernel_spmd

B = 262144
K = 128
NCORES = 8
BC = B // NCORES  # rows per core
P = 128           # partitions
NT = BC // P      # row-tiles per core (column index t in the [P, NT] view)
NBANK = 4         # PSUM banks rotated across row-tiles for matmul ILP

TPB = 16          # row-tiles per DMA'd big tile
HPB = TPB // 2    # row-tiles per exp/one-hot chunk (finer pipeline grain)

f32 = mybir.dt.float32
bf16 = mybir.dt.bfloat16

LAST_EXEC_NS = None
LAST_TRACE = None
LAST_PROFILE_JSON = None


def build_nc():
    """Build the per-core Bass program."""
    nc = bacc.Bacc("TRN2", target_bir_lowering=False)
    logits = nc.declare_dram_parameter("logits", [BC, K], f32, isOutput=False)
    labcols = nc.declare_dram_parameter("labcols", [P, NT], f32,
                                        isOutput=False)
    iotak = nc.declare_dram_parameter("iotak", [P, K], bf16, isOutput=False)
    out = nc.declare_dram_parameter("out", [P, NBANK * K], f32, isOutput=True)

    with tile.TileContext(nc) as tc:
        with (
            tc.tile_pool(name="const", bufs=1) as cpool,
            tc.tile_pool(name="lt", bufs=6) as ltpool,
            tc.tile_pool(name="ee", bufs=8) as epool,
            tc.tile_pool(name="oh", bufs=8) as ohpool,
            tc.tile_pool(name="psum", bufs=1, space="PSUM") as pspool,
        ):
            # first logits DMA goes out before the tiny const DMAs
            lg3 = logits.ap().rearrange("(p t) k -> p t k", p=P)
            lt_first = ltpool.tile([P, TPB * K], f32)
            nc.sync.dma_start(out=lt_first[:], in_=lg3[:, 0:TPB, :])

            labc = cpool.tile([P, NT], f32)
            nc.sync.dma_start(out=labc[:], in_=labcols.ap())
            iota = cpool.tile([P, K], bf16)
            nc.sync.dma_start(out=iota[:], in_=iotak.ap())

            psums = [pspool.tile([P, K], f32, name=f"ps{b}", tag=f"ps{b}")
                     for b in range(NBANK)]
            osb = cpool.tile([P, NBANK * K], f32)

            HW = HPB * K
            NSEG = NT // TPB
            for g in range(NSEG):
                t0 = g * TPB
                last = g == NSEG - 1
                if g == 0:
                    lt = lt_first
                elif not last:
                    lt = ltpool.tile([P, TPB * K], f32)
                    nc.sync.dma_start(out=lt[:], in_=lg3[:, t0:t0 + TPB, :])

                # chunked: finer exp grain keeps the matmul stream right
                # behind the DMA; the last seg uses quarter chunks, each
                # with its own sub-DMA, so the post-DMA drain is short
                cpb = HPB if not last else HPB // 2
                for h in range(TPB // cpb):
                    h0 = t0 + h * cpb
                    cw = cpb * K

                    if last:
                        lt = ltpool.tile([P, cw], f32)
                        nc.sync.dma_start(out=lt[:],
                                          in_=lg3[:, h0:h0 + cpb, :])
                        src_ap = lt[:]
                    else:
                        src_ap = lt[:, h * cw:(h + 1) * cw]

                    # E = exp(logits), cast to bf16
                    ee = epool.tile([P, cw], bf16)
                    nc.scalar.activation(
                        out=ee[:], in_=src_ap,
                        func=mybir.ActivationFunctionType.Exp)

                    # one-hot per row-tile: oh[p, k] = (label[p, t] == k)
                    # (single-source tensor_scalar -> fast DVE mode)
                    oh = ohpool.tile([P, cw], bf16)
                    for q in range(cpb):
                        t = h0 + q
                        nc.vector.tensor_scalar(
                            out=oh[:, q * K:(q + 1) * K], in0=iota[:],
                            scalar1=labc[:, t:t + 1], scalar2=None,
                            op0=mybir.AluOpType.is_equal)

                    # bin exp by label: S[c, k] += sum_p oh[p,c] * E[p,k]
                    for q in range(cpb):
                        t = h0 + q
                        b = t % NBANK
                        nc.tensor.matmul(
                            out=psums[b][:],
                            lhsT=oh[:, q * K:(q + 1) * K],
                            rhs=ee[:, q * K:(q + 1) * K],
                            start=(t < NBANK),
                            stop=(t >= NT - NBANK),
                        )

            for b in range(NBANK):
                if b % 2 == 0:
                    nc.vector.tensor_copy(osb[:, b * K:(b + 1) * K],
                                          psums[b][:])
                else:
                    nc.scalar.copy(osb[:, b * K:(b + 1) * K], psums[b][:])
            nc.sync.dma_start(out=out.ap(), in_=osb[:])

    nc.compile()
    return nc


def _shard_inputs(logits, labels):
    """Build the 8 per-core input maps (host-side layout only)."""
    logits = np.ascontiguousarray(np.asarray(logits, dtype=np.float32))
    labels = np.asarray(labels, dtype=np.int32)
    iota = np.broadcast_to(np.arange(K, dtype=np.float32), (P, K)).astype(
        ml_dtypes.bfloat16)
    in_maps = []
    for i in range(NCORES):
        sl = slice(i * BC, (i + 1) * BC)
        lab = labels[sl].astype(np.float32).reshape(P, NT)
        in_maps.append({
            "logits": logits[sl],
            "labcols": np.ascontiguousarray(lab),
            "iotak": iota,
        })
    return in_maps


def _finish(outs, logits, labels, events):
    """Host epilogue: all-reduce binned sums, triangular sum, numer/n_ev
    from 1-D data, the log, and the final scalar reduction."""
    labels = np.asarray(labels, dtype=np.int32)
    events = np.asarray(events, dtype=np.int32)
    S = np.zeros((P, K), dtype=np.float64)
    for o in outs:
        S += o.astype(np.float64).reshape(P, NBANK, K).sum(axis=1)
    # sumexp[k] = sum over label bins c >= k
    sumexp = (S * np.tri(K)).sum(axis=0)
    ev = events == 1
    own = np.asarray(logits)[np.arange(labels.shape[0]), labels].astype(
        np.float64)
    n_ev = np.bincount(labels[ev], minlength=K).astype(np.float64)
    numer = np.bincount(labels[ev], weights=own[ev], minlength=K)
    with np.errstate(divide="ignore"):
        denom_log = np.log(sumexp)
    terms = np.where(n_ev > 0, numer - n_ev * denom_log, 0.0)
    n_total = max(n_ev.sum(), 1.0)
    return np.array(-terms.sum() / n_total, dtype=np.float32)


def kernel(logits, labels, events, _trace=False):
    global LAST_EXEC_NS, LAST_TRACE, LAST_PROFILE_JSON
    in_maps = _shard_inputs(logits, labels)
    nc = build_nc()
    try:
        res = run_bass_kernel_spmd(nc, in_maps, core_ids=list(range(NCORES)),
                                   trace=_trace)
    except Exception:
        # one retry: absorbs transient NRT device-unrecoverable hiccups
        res = run_bass_kernel_spmd(nc, in_maps, core_ids=list(range(NCORES)),
                                   trace=_trace)
    LAST_EXEC_NS = res.exec_time_ns
    LAST_TRACE = res.instructions_and_trace
    LAST_PROFILE_JSON = res.profile_json
    outs = [res.results[i]["out"] for i in range(NCORES)]
    return _finish(outs, logits, labels, events)
